# revision 15
# baseline (speedup 1.0000x reference)
"""Multi-headed self-attention (B=8, S=1024, D=768, H=12) on 8 TRN2 cores.

Sharding: data-parallel over batch -- core i computes batch element i.

v3 (vs the fp32 baseline):
  - x and W shipped/projected in bf16 (halves input DMA: 10.6MB -> 5.3MB;
    PE still streams 1 col/cycle). q/k/scores/Et/V/PV stay fp32(r).
  - prologue reordered: Q-projection starts as soon as wq+x land, the
    first score units + exp start ~12us in (vs ~35us); the V projection
    and later Q/K projections are injected between attention units in
    3-matmul chunks so the scalar engine's exp pipeline never backs up
    far enough to stall the tensor engine on score-psum reuse.
Per-core kernel (all operands pre-transposed on host):
    Qt = (Wq @ x.T + bq)      [D, S]   (o on partitions)
    Kt = (Wk @ x.T + bk)      [D, S]
    V  = (x @ Wv.T + bv)      [S, D]   augmented with a ones column per head
    St_h = Kt_h^T-slices @ Qt_h   -> scores transposed [k, q]
    Et = exp(St/8 + maskbias[k])  (ACT, mask bias per-partition)
    PVt'_h = V'_h.T @ Et_h        [65, q]; row 64 = sum_k Et = Z[q]
    out_h.T = PVt'_h[0:64] / Z    -> outT rows h*64..h*64+63
Host transposes outT back.
"""

import numpy as np
import ml_dtypes

import concourse.bacc as bacc
import concourse.tile as tile
from concourse import mybir
from concourse.bass_utils import run_bass_kernel_spmd

B, S, D, H = 8, 1024, 768, 12
HD = D // H  # 64
N_CORES = 8
SC = S // 128  # 8 key/seq chunks
OC = D // 128  # 6 output chunks (2 heads each)
DC = D // 128  # 6 contraction chunks
NT = 512  # matmul moving-dim tile (fp32 max)
QT = S // NT  # 2
F32 = mybir.dt.float32
F32R = mybir.dt.float32r
BF16 = mybir.dt.bfloat16

HW = HD + 1  # per-head V width incl. ones column


def build():
    nc = bacc.Bacc("TRN2", target_bir_lowering=False, debug=False, num_devices=N_CORES)
    xT = nc.dram_tensor("xT", [D, S], BF16, kind="ExternalInput").ap()
    wqT = nc.dram_tensor("wqT", [D, D], BF16, kind="ExternalInput").ap()
    wkT = nc.dram_tensor("wkT", [D, D], BF16, kind="ExternalInput").ap()
    wvT = nc.dram_tensor("wvT", [D, D], BF16, kind="ExternalInput").ap()
    bq = nc.dram_tensor("bq", [D], F32, kind="ExternalInput").ap()
    bk = nc.dram_tensor("bk", [D], F32, kind="ExternalInput").ap()
    bvb = nc.dram_tensor("bvb", [128, D], F32, kind="ExternalInput").ap()
    mb = nc.dram_tensor("mb", [S], F32, kind="ExternalInput").ap()
    outT = nc.dram_tensor("outT", [D, S], F32, kind="ExternalOutput").ap()

    with tile.TileContext(nc) as tc:
        with (
            tc.tile_pool(name="const", bufs=1) as const,
            tc.tile_pool(name="qk", bufs=2) as qk_pool,
            tc.tile_pool(name="et", bufs=6) as et_pool,
            tc.tile_pool(name="epi", bufs=2) as epi_pool,
            tc.tile_pool(name="st", bufs=3, space="PSUM") as st_ps,
            tc.tile_pool(name="pv", bufs=2, space="PSUM") as pv_ps,
            tc.tile_pool(name="dram", bufs=2, space="DRAM") as dram_pool,
        ):
            # ---------- constant / weight loads ----------
            mb_t = const.tile([128, SC], F32, tag="mb")
            nc.sync.dma_start(mb_t[:], mb.rearrange("(c p) -> p c", p=128))
            bq_t = const.tile([128, OC], F32, tag="bq")
            nc.sync.dma_start(bq_t[:], bq.rearrange("(c p) -> p c", p=128))
            bk_t = const.tile([128, OC], F32, tag="bk")
            nc.sync.dma_start(bk_t[:], bk.rearrange("(c p) -> p c", p=128))
            bvb_t = const.tile([128, D], F32, tag="bvb")
            nc.sync.dma_start(bvb_t[:], bvb[:])

            xt = [const.tile([128, S], BF16, tag=f"xt{c}", name=f"xt{c}") for c in range(DC)]
            wq = [const.tile([128, D], BF16, tag=f"wq{c}", name=f"wq{c}") for c in range(DC)]
            wk = [const.tile([128, D], BF16, tag=f"wk{c}", name=f"wk{c}") for c in range(DC)]
            wv = [const.tile([128, D], BF16, tag=f"wv{c}", name=f"wv{c}") for c in range(DC)]
            # DMA triggers cost ~0.6us each on the issuing engine's queue,
            # so spread the input streams across four otherwise-idle
            # queues: x on gpsimd, wq-oc0 slices on sync (first Q matmuls
            # need wq-oc0 + x only), wk-oc0 + wv on scalar, and the
            # wq/wk remainders on vector.
            for c in range(DC):
                nc.sync.dma_start(wq[c][:, 0:128], wqT[c * 128:(c + 1) * 128, 0:128])
                nc.gpsimd.dma_start(xt[c][:], xT[c * 128:(c + 1) * 128, :])
            for c in range(DC):
                nc.scalar.dma_start(wk[c][:, 0:128], wkT[c * 128:(c + 1) * 128, 0:128])
            for c in range(DC):
                nc.scalar.dma_start(wv[c][:], wvT[c * 128:(c + 1) * 128, :])
            for c in range(DC):
                nc.gpsimd.dma_start(wq[c][:, 128:D], wqT[c * 128:(c + 1) * 128, 128:D])
            for c in range(DC):
                nc.gpsimd.dma_start(wk[c][:, 128:D], wkT[c * 128:(c + 1) * 128, 128:D])
            # tiny dummy exp pulls the ~2.7us ACT table load off the
            # critical path (walrus emits the table load before the first
            # ACTIVATE in queue order)
            warm = const.tile([128, 1], F32, tag="warm")
            nc.scalar.activation(
                warm[:], mb_t[:, 0:1], mybir.ActivationFunctionType.Exp
            )

            # ---------- V storage [sc][128, H*65] fp32r ----------
            vaug = [const.tile([128, H * HW], BF16, tag=f"va{sc}", name=f"va{sc}") for sc in range(SC)]
            for sc in range(SC):
                ones_cols = vaug[sc][:].rearrange("p (h w) -> p h w", h=H)[:, :, HD:HW]
                nc.vector.memset(ones_cols, 1.0)

            # V projection piece, split into two 3-matmul chunks for
            # fine-grained injection between attention units
            def v_chunks(sc, half):
                n0, n1, h0, h1 = ((0, 512, 0, 8), (512, 768, 8, 12))[half]
                box = {}

                def c0():
                    box["vp"] = st_ps.tile([128, NT], F32, tag="st", name=f"vp{sc}_{half}")
                    for c in range(3):
                        nc.tensor.matmul(
                            box["vp"][:, : n1 - n0],
                            xt[c][:, sc * 128:(sc + 1) * 128],
                            wv[c][:, n0:n1],
                            start=(c == 0),
                            stop=False,
                            skip_group_check=True,
                        )

                def c1():
                    vp = box["vp"]
                    for c in range(3, DC):
                        nc.tensor.matmul(
                            vp[:, : n1 - n0],
                            xt[c][:, sc * 128:(sc + 1) * 128],
                            wv[c][:, n0:n1],
                            start=False,
                            stop=(c == DC - 1),
                            skip_group_check=True,
                        )
                    nc.vector.tensor_add(
                        vaug[sc][:].rearrange("p (h w) -> p h w", h=H)[:, h0:h1, 0:HD],
                        vp[:, : n1 - n0].rearrange("p (h w) -> p h w", w=HD),
                        bvb_t[:, n0:n1].rearrange("p (h w) -> p h w", w=HD),
                    )

                return [c0, c1]

            # ---------- Q/K projection ----------
            wmap = {"q": (wq, bq_t), "k": (wk, bk_t)}

            def qk_alloc(oc):
                return {
                    name: qk_pool.tile([128, S], BF16, tag=name, name=f"{name}t{oc}")
                    for name in ("q", "k")
                }

            def qk_chunks(oc, dsts, name, qt):
                w_t, b_t = wmap[name]
                box = {}

                def c0():
                    box["p"] = st_ps.tile([128, NT], F32, tag="st", name=f"qkp{name}{qt}")
                    for c in range(3):
                        nc.tensor.matmul(
                            box["p"][:],
                            w_t[c][:, oc * 128:(oc + 1) * 128],
                            xt[c][:, qt * NT:(qt + 1) * NT],
                            start=(c == 0),
                            stop=False,
                            skip_group_check=True,
                        )

                def c1():
                    p = box["p"]
                    for c in range(3, DC):
                        nc.tensor.matmul(
                            p[:],
                            w_t[c][:, oc * 128:(oc + 1) * 128],
                            xt[c][:, qt * NT:(qt + 1) * NT],
                            start=False,
                            stop=(c == DC - 1),
                            skip_group_check=True,
                        )
                    nc.vector.tensor_scalar_add(
                        dsts[name][:, qt * NT:(qt + 1) * NT], p[:], b_t[:, oc:oc + 1]
                    )

                return [c0, c1]

            def qk_proj(oc):
                dsts = qk_alloc(oc)
                for name in ("q", "k"):
                    for qt in range(QT):
                        for ch in qk_chunks(oc, dsts, name, qt):
                            ch()
                return dsts

            # ---------- attention: flat software pipeline, skew=2 ----------
            qkts = {0: qk_proj(0)}
            units = [(oc, hh, kc) for oc in range(OC) for hh in range(2)
                     for kc in range(SC)]
            NU = len(units)
            SKEW = 2
            st_tiles = {}
            pvq_map = {}

            def emit_scores(i):
                oc, hh, kc = units[i]
                p0 = hh * 64
                qkt = qkts[oc]
                stt = st_ps.tile([128, S], F32, tag="st", name=f"st{i}")
                for qt in range(QT):
                    nc.tensor.matmul(
                        stt[:, qt * NT:(qt + 1) * NT],
                        qkt["k"][p0:p0 + 64, kc * 128:(kc + 1) * 128],
                        qkt["q"][p0:p0 + 64, qt * NT:(qt + 1) * NT],
                        tile_position=(p0, 0),
                    )
                st_tiles[i] = stt

            def emit_epilogue(oc, hh):
                gh = 2 * oc + hh
                pvq = pvq_map.pop((oc, hh))
                pvs = epi_pool.tile([HW, S], F32, tag="pvs", name="pvs", bufs=3)
                for qt in range(QT):
                    nc.vector.tensor_copy(
                        pvs[:, qt * NT:(qt + 1) * NT], pvq[qt][:]
                    )
                # Z row -> DRAM -> partition-broadcast read, then DVE
                # reciprocal + multiply (3 cross-engine hops total; DMAs
                # ride the sync queue).
                zd = dram_pool.tile([S], F32, tag="zd", name="zd", bufs=4)
                nc.sync.dma_start(zd.rearrange("(o s) -> o s", o=1), pvs[HD:HW, :])
                zb = epi_pool.tile([HD, S], F32, tag="zb", name="zb", bufs=3)
                nc.sync.dma_start(zb[:], zd[:].partition_broadcast(HD))
                nc.vector.reciprocal(zb[:], zb[:])
                oh = epi_pool.tile([HD, S], F32, tag="oh", name="oh", bufs=3)
                nc.vector.tensor_mul(oh[:], pvs[0:HD, :], zb[:])
                nc.sync.dma_start(outT[gh * HD:(gh + 1) * HD, :], oh[:])

            # pre-roll: two score units, then V chunks for sc 0..1 so
            # exp(0)/exp(1) overlap the V projection start
            for i in range(SKEW):
                emit_scores(i)
            for sc in (0, 1):
                for half in (0, 1):
                    for ch in v_chunks(sc, half):
                        ch()

            # injection queue of small tensor-work chunks
            from collections import deque
            queue = deque()
            for sc in range(2, SC):
                for half in (0, 1):
                    queue.extend(v_chunks(sc, half))
            # rates[i] = how many chunks to inject after unit i
            rates = [0] * NU
            # 24 V chunks over units 0..5 (deadline: vaug[kc] before unit kc,
            # sc=2+j fully injected by end of unit j)
            for j in range(6):
                rates[j] += 4
            # Q/K proj for oc+1: 8 chunks over units (oc,0,6)..(oc,1,5) --
            # as late as the scores(+SKEW) deadline allows, to cover the
            # otherwise scalar-paced stretches
            qk_sched = {}
            for oc in range(OC - 1):
                base = oc * 16 + 6
                for j in range(8):
                    qk_sched.setdefault(base + j, []).append(oc + 1)
                    rates[base + j] += 1

            for i, (oc, hh, kc) in enumerate(units):
                if i + SKEW < NU:
                    emit_scores(i + SKEW)
                stt = st_tiles.pop(i)
                ett = et_pool.tile([128, S], BF16, tag="et", name=f"et{i}")
                nc.scalar.activation(
                    ett[:],
                    stt[:],
                    mybir.ActivationFunctionType.Exp,
                    bias=mb_t[:, kc:kc + 1],
                    scale=1.0 / np.sqrt(HD),
                )
                gh = 2 * oc + hh
                if kc == 0:
                    pvq_map[(oc, hh)] = [
                        pv_ps.tile([HW, NT], F32, tag="pv", name=f"pv{gh}_{qt}")
                        for qt in range(QT)
                    ]
                pvq = pvq_map[(oc, hh)]
                for qt in range(QT):
                    nc.tensor.matmul(
                        pvq[qt][:],
                        vaug[kc][:, gh * HW:(gh + 1) * HW],
                        ett[:, qt * NT:(qt + 1) * NT],
                        start=(kc == 0),
                        stop=(kc == SC - 1),
                    )
                if kc == SC - 1:
                    emit_epilogue(oc, hh)
                # inject queued projection chunks
                for oc_next in qk_sched.get(i, []):
                    if oc_next not in qkts:
                        qkts[oc_next] = qk_alloc(oc_next)
                        qkts.pop(oc_next - 2, None)
                        for name in ("q", "k"):
                            for qt in range(QT):
                                queue.extend(
                                    qk_chunks(oc_next, qkts[oc_next], name, qt)
                                )
                n = rates[i]
                while n > 0 and queue:
                    queue.popleft()()
                    n -= 1
            while queue:
                queue.popleft()()

    nc.compile()
    return nc


_NC = None


def _get_nc():
    global _NC
    if _NC is None:
        _NC = build()
    return _NC


def _in_maps(x, mask, Wq, bq, Wk, bk, Wv, bv):
    x = np.asarray(x, dtype=np.float32)
    mask = np.asarray(mask)
    bf = ml_dtypes.bfloat16
    wqT = np.ascontiguousarray(np.asarray(Wq, dtype=np.float32).T.astype(bf))
    wkT = np.ascontiguousarray(np.asarray(Wk, dtype=np.float32).T.astype(bf))
    wvT = np.ascontiguousarray(np.asarray(Wv, dtype=np.float32).T.astype(bf))
    bq = np.asarray(bq, dtype=np.float32)
    bk = np.asarray(bk, dtype=np.float32)
    bvb = np.ascontiguousarray(
        np.broadcast_to(np.asarray(bv, dtype=np.float32), (128, D))
    )
    maps = []
    for c in range(N_CORES):
        maps.append(
            {
                "xT": np.ascontiguousarray(x[c].T.astype(bf)),
                "wqT": wqT,
                "wkT": wkT,
                "wvT": wvT,
                "bq": bq,
                "bk": bk,
                "bvb": bvb,
                "mb": (-10000.0 * (1.0 - mask[c].astype(np.float32))).astype(
                    np.float32
                ),
            }
        )
    return maps


def run(inputs, trace=False, **kw):
    nc = _get_nc()
    res = run_bass_kernel_spmd(
        nc, _in_maps(**inputs), list(range(N_CORES)), trace=trace, **kw
    )
    out = np.stack(
        [np.ascontiguousarray(res.results[c]["outT"].T) for c in range(N_CORES)]
    ).astype(np.float32)
    return out, res


def kernel(**inputs):
    out, _ = run(inputs)
    return out


# revision 17
# speedup vs baseline: 1.3720x; 1.3720x over previous
"""Multi-headed self-attention (B=8, S=1024, D=768, H=12) on 8 TRN2 cores.

Sharding: data-parallel over batch -- core i computes batch element i.

v3 (vs the fp32 baseline):
  - x and W shipped/projected in bf16 (halves input DMA: 10.6MB -> 5.3MB;
    PE still streams 1 col/cycle). q/k/scores/Et/V/PV stay fp32(r).
  - prologue reordered: Q-projection starts as soon as wq+x land, the
    first score units + exp start ~12us in (vs ~35us); the V projection
    and later Q/K projections are injected between attention units in
    3-matmul chunks so the scalar engine's exp pipeline never backs up
    far enough to stall the tensor engine on score-psum reuse.
Per-core kernel (all operands pre-transposed on host):
    Qt = (Wq @ x.T + bq)      [D, S]   (o on partitions)
    Kt = (Wk @ x.T + bk)      [D, S]
    V  = (x @ Wv.T + bv)      [S, D]   augmented with a ones column per head
    St_h = Kt_h^T-slices @ Qt_h   -> scores transposed [k, q]
    Et = exp(St/8 + maskbias[k])  (ACT, mask bias per-partition)
    PVt'_h = V'_h.T @ Et_h        [65, q]; row 64 = sum_k Et = Z[q]
    out_h.T = PVt'_h[0:64] / Z    -> outT rows h*64..h*64+63
Host transposes outT back.
"""

import numpy as np
import ml_dtypes

import concourse.bacc as bacc
import concourse.tile as tile
from concourse import mybir
from concourse.bass_utils import run_bass_kernel_spmd

B, S, D, H = 8, 1024, 768, 12
HD = D // H  # 64
N_CORES = 8
SC = S // 128  # 8 key/seq chunks
OC = D // 128  # 6 output chunks (2 heads each)
DC = D // 128  # 6 contraction chunks
NT = 512  # matmul moving-dim tile (fp32 max)
QT = S // NT  # 2
F32 = mybir.dt.float32
F32R = mybir.dt.float32r
BF16 = mybir.dt.bfloat16

HW = HD + 1  # per-head V width incl. ones column


def build():
    nc = bacc.Bacc("TRN2", target_bir_lowering=False, debug=False, num_devices=N_CORES)
    xT = nc.dram_tensor("xT", [D, S], BF16, kind="ExternalInput").ap()
    wqT = nc.dram_tensor("wqT", [D, D], BF16, kind="ExternalInput").ap()
    wkT = nc.dram_tensor("wkT", [D, D], BF16, kind="ExternalInput").ap()
    wvT = nc.dram_tensor("wvT", [D, D], BF16, kind="ExternalInput").ap()
    bq = nc.dram_tensor("bq", [D], F32, kind="ExternalInput").ap()
    bk = nc.dram_tensor("bk", [D], F32, kind="ExternalInput").ap()
    bvb = nc.dram_tensor("bvb", [128, D], F32, kind="ExternalInput").ap()
    mb = nc.dram_tensor("mb", [S], F32, kind="ExternalInput").ap()
    outT = nc.dram_tensor("outT", [D, S], F32, kind="ExternalOutput").ap()

    with tile.TileContext(nc) as tc:
        with (
            tc.tile_pool(name="const", bufs=1) as const,
            tc.tile_pool(name="qk", bufs=2) as qk_pool,
            tc.tile_pool(name="et", bufs=6) as et_pool,
            tc.tile_pool(name="epi", bufs=2) as epi_pool,
            tc.tile_pool(name="st", bufs=3, space="PSUM") as st_ps,
            tc.tile_pool(name="pv", bufs=2, space="PSUM") as pv_ps,
            tc.tile_pool(name="dram", bufs=2, space="DRAM") as dram_pool,
        ):
            # ---------- constant / weight loads ----------
            mb_t = const.tile([128, SC], F32, tag="mb")
            nc.sync.dma_start(mb_t[:], mb.rearrange("(c p) -> p c", p=128))
            bq_t = const.tile([128, OC], F32, tag="bq")
            nc.sync.dma_start(bq_t[:], bq.rearrange("(c p) -> p c", p=128))
            bk_t = const.tile([128, OC], F32, tag="bk")
            nc.sync.dma_start(bk_t[:], bk.rearrange("(c p) -> p c", p=128))
            bvb_t = const.tile([128, D], F32, tag="bvb")
            nc.sync.dma_start(bvb_t[:], bvb[:])

            xt = [const.tile([128, S], BF16, tag=f"xt{c}", name=f"xt{c}") for c in range(DC)]
            wq = [const.tile([128, D], BF16, tag=f"wq{c}", name=f"wq{c}") for c in range(DC)]
            wk = [const.tile([128, D], BF16, tag=f"wk{c}", name=f"wk{c}") for c in range(DC)]
            wv = [const.tile([128, D], BF16, tag=f"wv{c}", name=f"wv{c}") for c in range(DC)]
            # DMA triggers cost ~0.6us each on the issuing engine's queue,
            # so split the input streams across the two idle DMA-capable
            # queues: sync carries the critical path for the first Q
            # matmuls (wq-oc0 slices + x), gpsimd carries the rest.
            # (Scalar can trigger DMAs too but that would delay the exps.)
            for c in range(DC):
                nc.sync.dma_start(wq[c][:, 0:128], wqT[c * 128:(c + 1) * 128, 0:128])
                nc.sync.dma_start(xt[c][:], xT[c * 128:(c + 1) * 128, :])
            for c in range(DC):
                nc.gpsimd.dma_start(wk[c][:, 0:128], wkT[c * 128:(c + 1) * 128, 0:128])
            for c in range(DC):
                nc.gpsimd.dma_start(wv[c][:], wvT[c * 128:(c + 1) * 128, :])
            for c in range(DC):
                nc.gpsimd.dma_start(wq[c][:, 128:D], wqT[c * 128:(c + 1) * 128, 128:D])
            for c in range(DC):
                nc.gpsimd.dma_start(wk[c][:, 128:D], wkT[c * 128:(c + 1) * 128, 128:D])
            # tiny dummy exp pulls the ~2.7us ACT table load off the
            # critical path (walrus emits the table load before the first
            # ACTIVATE in queue order)
            warm = const.tile([128, 1], F32, tag="warm")
            nc.scalar.activation(
                warm[:], mb_t[:, 0:1], mybir.ActivationFunctionType.Exp
            )

            # ---------- V storage [sc][128, H*65] fp32r ----------
            vaug = [const.tile([128, H * HW], BF16, tag=f"va{sc}", name=f"va{sc}") for sc in range(SC)]
            for sc in range(SC):
                ones_cols = vaug[sc][:].rearrange("p (h w) -> p h w", h=H)[:, :, HD:HW]
                nc.vector.memset(ones_cols, 1.0)

            # V projection piece, split into two 3-matmul chunks for
            # fine-grained injection between attention units
            def v_chunks(sc, half):
                n0, n1, h0, h1 = ((0, 512, 0, 8), (512, 768, 8, 12))[half]
                box = {}

                def c0():
                    box["vp"] = st_ps.tile([128, NT], F32, tag="st", name=f"vp{sc}_{half}")
                    for c in range(3):
                        nc.tensor.matmul(
                            box["vp"][:, : n1 - n0],
                            xt[c][:, sc * 128:(sc + 1) * 128],
                            wv[c][:, n0:n1],
                            start=(c == 0),
                            stop=False,
                            skip_group_check=True,
                        )

                def c1():
                    vp = box["vp"]
                    for c in range(3, DC):
                        nc.tensor.matmul(
                            vp[:, : n1 - n0],
                            xt[c][:, sc * 128:(sc + 1) * 128],
                            wv[c][:, n0:n1],
                            start=False,
                            stop=(c == DC - 1),
                            skip_group_check=True,
                        )
                    nc.vector.tensor_add(
                        vaug[sc][:].rearrange("p (h w) -> p h w", h=H)[:, h0:h1, 0:HD],
                        vp[:, : n1 - n0].rearrange("p (h w) -> p h w", w=HD),
                        bvb_t[:, n0:n1].rearrange("p (h w) -> p h w", w=HD),
                    )

                return [c0, c1]

            # ---------- Q/K projection ----------
            wmap = {"q": (wq, bq_t), "k": (wk, bk_t)}

            def qk_alloc(oc):
                return {
                    name: qk_pool.tile([128, S], BF16, tag=name, name=f"{name}t{oc}")
                    for name in ("q", "k")
                }

            def qk_chunks(oc, dsts, name, qt):
                w_t, b_t = wmap[name]
                box = {}

                def c0():
                    box["p"] = st_ps.tile([128, NT], F32, tag="st", name=f"qkp{name}{qt}")
                    for c in range(3):
                        nc.tensor.matmul(
                            box["p"][:],
                            w_t[c][:, oc * 128:(oc + 1) * 128],
                            xt[c][:, qt * NT:(qt + 1) * NT],
                            start=(c == 0),
                            stop=False,
                            skip_group_check=True,
                        )

                def c1():
                    p = box["p"]
                    for c in range(3, DC):
                        nc.tensor.matmul(
                            p[:],
                            w_t[c][:, oc * 128:(oc + 1) * 128],
                            xt[c][:, qt * NT:(qt + 1) * NT],
                            start=False,
                            stop=(c == DC - 1),
                            skip_group_check=True,
                        )
                    nc.vector.tensor_scalar_add(
                        dsts[name][:, qt * NT:(qt + 1) * NT], p[:], b_t[:, oc:oc + 1]
                    )

                return [c0, c1]

            def qk_proj(oc):
                dsts = qk_alloc(oc)
                for name in ("q", "k"):
                    for qt in range(QT):
                        for ch in qk_chunks(oc, dsts, name, qt):
                            ch()
                return dsts

            # ---------- attention: flat software pipeline, skew=2 ----------
            qkts = {0: qk_proj(0)}
            units = [(oc, hh, kc) for oc in range(OC) for hh in range(2)
                     for kc in range(SC)]
            NU = len(units)
            SKEW = 2
            st_tiles = {}
            pvq_map = {}

            def emit_scores(i):
                oc, hh, kc = units[i]
                p0 = hh * 64
                qkt = qkts[oc]
                stt = st_ps.tile([128, S], F32, tag="st", name=f"st{i}")
                for qt in range(QT):
                    nc.tensor.matmul(
                        stt[:, qt * NT:(qt + 1) * NT],
                        qkt["k"][p0:p0 + 64, kc * 128:(kc + 1) * 128],
                        qkt["q"][p0:p0 + 64, qt * NT:(qt + 1) * NT],
                        tile_position=(p0, 0),
                    )
                st_tiles[i] = stt

            def emit_epilogue(oc, hh):
                gh = 2 * oc + hh
                pvq = pvq_map.pop((oc, hh))
                pvs = epi_pool.tile([HW, S], F32, tag="pvs", name="pvs", bufs=3)
                for qt in range(QT):
                    nc.vector.tensor_copy(
                        pvs[:, qt * NT:(qt + 1) * NT], pvq[qt][:]
                    )
                # Z row -> [128, 8] partition-scatter (p-major), reciprocal,
                # bounce through DRAM for the partition-broadcast read.
                # All DMAs ride the sync queue (no gpsimd semaphore hops).
                zp = epi_pool.tile([128, SC], F32, tag="zp", name="zp", bufs=4)
                nc.sync.dma_start(
                    zp[:], pvs[HD:HW, :].rearrange("o (p c) -> o p c", c=SC)
                )
                nc.vector.reciprocal(zp[:], zp[:])
                rzd = dram_pool.tile([S], F32, tag="rzd", name="rzd", bufs=4)
                nc.sync.dma_start(rzd.rearrange("(p c) -> p c", c=SC), zp[:])
                zb = epi_pool.tile([HD, S], F32, tag="zb", name="zb", bufs=3)
                nc.sync.dma_start(zb[:], rzd[:].partition_broadcast(HD))
                oh = epi_pool.tile([HD, S], F32, tag="oh", name="oh", bufs=3)
                nc.vector.tensor_mul(oh[:], pvs[0:HD, :], zb[:])
                nc.sync.dma_start(outT[gh * HD:(gh + 1) * HD, :], oh[:])

            # pre-roll: two score units, then V chunks for sc 0..1 so
            # exp(0)/exp(1) overlap the V projection start
            for i in range(SKEW):
                emit_scores(i)
            for sc in (0, 1):
                for half in (0, 1):
                    for ch in v_chunks(sc, half):
                        ch()

            # injection queue of small tensor-work chunks
            from collections import deque
            queue = deque()
            for sc in range(2, SC):
                for half in (0, 1):
                    queue.extend(v_chunks(sc, half))
            # rates[i] = how many chunks to inject after unit i
            rates = [0] * NU
            # 24 V chunks over units 0..5 (deadline: vaug[kc] before unit kc,
            # sc=2+j fully injected by end of unit j)
            for j in range(6):
                rates[j] += 4
            # Q/K proj for oc+1: 8 chunks over units (oc,0,6)..(oc,1,5) --
            # as late as the scores(+SKEW) deadline allows, to cover the
            # otherwise scalar-paced stretches
            qk_sched = {}
            for oc in range(OC - 1):
                base = oc * 16 + 6
                for j in range(8):
                    qk_sched.setdefault(base + j, []).append(oc + 1)
                    rates[base + j] += 1

            for i, (oc, hh, kc) in enumerate(units):
                if i + SKEW < NU:
                    emit_scores(i + SKEW)
                stt = st_tiles.pop(i)
                ett = et_pool.tile([128, S], BF16, tag="et", name=f"et{i}")
                nc.scalar.activation(
                    ett[:],
                    stt[:],
                    mybir.ActivationFunctionType.Exp,
                    bias=mb_t[:, kc:kc + 1],
                    scale=1.0 / np.sqrt(HD),
                )
                gh = 2 * oc + hh
                if kc == 0:
                    pvq_map[(oc, hh)] = [
                        pv_ps.tile([HW, NT], F32, tag="pv", name=f"pv{gh}_{qt}")
                        for qt in range(QT)
                    ]
                pvq = pvq_map[(oc, hh)]
                for qt in range(QT):
                    nc.tensor.matmul(
                        pvq[qt][:],
                        vaug[kc][:, gh * HW:(gh + 1) * HW],
                        ett[:, qt * NT:(qt + 1) * NT],
                        start=(kc == 0),
                        stop=(kc == SC - 1),
                    )
                if kc == SC - 1:
                    emit_epilogue(oc, hh)
                # inject queued projection chunks
                for oc_next in qk_sched.get(i, []):
                    if oc_next not in qkts:
                        qkts[oc_next] = qk_alloc(oc_next)
                        qkts.pop(oc_next - 2, None)
                        for name in ("q", "k"):
                            for qt in range(QT):
                                queue.extend(
                                    qk_chunks(oc_next, qkts[oc_next], name, qt)
                                )
                n = rates[i]
                while n > 0 and queue:
                    queue.popleft()()
                    n -= 1
            while queue:
                queue.popleft()()

    nc.compile()
    return nc


_NC = None


def _get_nc():
    global _NC
    if _NC is None:
        _NC = build()
    return _NC


def _in_maps(x, mask, Wq, bq, Wk, bk, Wv, bv):
    x = np.asarray(x, dtype=np.float32)
    mask = np.asarray(mask)
    bf = ml_dtypes.bfloat16
    wqT = np.ascontiguousarray(np.asarray(Wq, dtype=np.float32).T.astype(bf))
    wkT = np.ascontiguousarray(np.asarray(Wk, dtype=np.float32).T.astype(bf))
    wvT = np.ascontiguousarray(np.asarray(Wv, dtype=np.float32).T.astype(bf))
    bq = np.asarray(bq, dtype=np.float32)
    bk = np.asarray(bk, dtype=np.float32)
    bvb = np.ascontiguousarray(
        np.broadcast_to(np.asarray(bv, dtype=np.float32), (128, D))
    )
    maps = []
    for c in range(N_CORES):
        maps.append(
            {
                "xT": np.ascontiguousarray(x[c].T.astype(bf)),
                "wqT": wqT,
                "wkT": wkT,
                "wvT": wvT,
                "bq": bq,
                "bk": bk,
                "bvb": bvb,
                "mb": (-10000.0 * (1.0 - mask[c].astype(np.float32))).astype(
                    np.float32
                ),
            }
        )
    return maps


def run(inputs, trace=False, **kw):
    nc = _get_nc()
    res = run_bass_kernel_spmd(
        nc, _in_maps(**inputs), list(range(N_CORES)), trace=trace, **kw
    )
    out = np.stack(
        [np.ascontiguousarray(res.results[c]["outT"].T) for c in range(N_CORES)]
    ).astype(np.float32)
    return out, res


def kernel(**inputs):
    out, _ = run(inputs)
    return out


# revision 27
# speedup vs baseline: 1.4183x; 1.0338x over previous
"""Multi-headed self-attention (B=8, S=1024, D=768, H=12) on 8 TRN2 cores.

Sharding: data-parallel over batch -- core i computes batch element i.

v3 (vs the fp32 baseline):
  - x and W shipped/projected in bf16 (halves input DMA: 10.6MB -> 5.3MB;
    PE still streams 1 col/cycle). q/k/scores/Et/V/PV stay fp32(r).
  - prologue reordered: Q-projection starts as soon as wq+x land, the
    first score units + exp start ~12us in (vs ~35us); the V projection
    and later Q/K projections are injected between attention units in
    3-matmul chunks so the scalar engine's exp pipeline never backs up
    far enough to stall the tensor engine on score-psum reuse.
Per-core kernel (all operands pre-transposed on host):
    Qt = (Wq @ x.T + bq)      [D, S]   (o on partitions)
    Kt = (Wk @ x.T + bk)      [D, S]
    V  = (x @ Wv.T + bv)      [S, D]   augmented with a ones column per head
    St_h = Kt_h^T-slices @ Qt_h   -> scores transposed [k, q]
    Et = exp(St/8 + maskbias[k])  (ACT, mask bias per-partition)
    PVt'_h = V'_h.T @ Et_h        [65, q]; row 64 = sum_k Et = Z[q]
    out_h.T = PVt'_h[0:64] / Z    -> outT rows h*64..h*64+63
Host transposes outT back.
"""

import numpy as np
import ml_dtypes

import concourse.bacc as bacc
import concourse.tile as tile
from concourse import mybir
from concourse.bass_utils import run_bass_kernel_spmd

B, S, D, H = 8, 1024, 768, 12
HD = D // H  # 64
N_CORES = 8
SC = S // 128  # 8 key/seq chunks
OC = D // 128  # 6 output chunks (2 heads each)
DC = D // 128  # 6 contraction chunks
NT = 512  # matmul moving-dim tile (fp32 max)
QT = S // NT  # 2
F32 = mybir.dt.float32
F32R = mybir.dt.float32r
BF16 = mybir.dt.bfloat16

HW = HD + 1  # per-head V width incl. ones column


def build():
    nc = bacc.Bacc("TRN2", target_bir_lowering=False, debug=False, num_devices=N_CORES)
    xT = nc.dram_tensor("xT", [D, S], BF16, kind="ExternalInput").ap()
    wqT = nc.dram_tensor("wqT", [D, D], BF16, kind="ExternalInput").ap()
    wkT = nc.dram_tensor("wkT", [D, D], BF16, kind="ExternalInput").ap()
    wvT = nc.dram_tensor("wvT", [D, D], BF16, kind="ExternalInput").ap()
    bq = nc.dram_tensor("bq", [D], F32, kind="ExternalInput").ap()
    bk = nc.dram_tensor("bk", [D], F32, kind="ExternalInput").ap()
    bvb = nc.dram_tensor("bvb", [128, D], F32, kind="ExternalInput").ap()
    mb = nc.dram_tensor("mb", [S], F32, kind="ExternalInput").ap()
    outT = nc.dram_tensor("outT", [D, S], F32, kind="ExternalOutput").ap()

    with tile.TileContext(nc) as tc:
        with (
            tc.tile_pool(name="const", bufs=1) as const,
            tc.tile_pool(name="qk", bufs=2) as qk_pool,
            tc.tile_pool(name="et", bufs=6) as et_pool,
            tc.tile_pool(name="epi", bufs=2) as epi_pool,
            tc.tile_pool(name="st", bufs=3, space="PSUM") as st_ps,
            tc.tile_pool(name="pv", bufs=2, space="PSUM") as pv_ps,
            tc.tile_pool(name="dram", bufs=2, space="DRAM") as dram_pool,
        ):
            # ---------- constant / weight loads ----------
            # DMA triggers cost ~0.6us each on the issuing engine's queue,
            # so split the input streams across the two idle DMA-capable
            # queues: sync carries the critical path for the first Q
            # matmuls (wq-oc0 slices + x), gpsimd carries the rest.
            # (Scalar can trigger DMAs too but that would delay the exps.)
            mb_t = const.tile([128, SC], F32, tag="mb")
            bq_t = const.tile([128, OC], F32, tag="bq")
            bk_t = const.tile([128, OC], F32, tag="bk")
            bvb_t = const.tile([128, D], F32, tag="bvb")
            xt = [const.tile([128, S], BF16, tag=f"xt{c}", name=f"xt{c}") for c in range(DC)]
            wq = [const.tile([128, D], BF16, tag=f"wq{c}", name=f"wq{c}") for c in range(DC)]
            wk = [const.tile([128, D], BF16, tag=f"wk{c}", name=f"wk{c}") for c in range(DC)]
            wv = [const.tile([128, D], BF16, tag=f"wv{c}", name=f"wv{c}") for c in range(DC)]
            for c in range(DC):
                nc.sync.dma_start(wq[c][:, 0:128], wqT[c * 128:(c + 1) * 128, 0:128])
                nc.sync.dma_start(xt[c][:], xT[c * 128:(c + 1) * 128, :])
            nc.sync.dma_start(bvb_t[:], bvb[:])
            nc.gpsimd.dma_start(mb_t[:], mb.rearrange("(c p) -> p c", p=128))
            for c in range(DC):
                nc.gpsimd.dma_start(wk[c][:, 0:128], wkT[c * 128:(c + 1) * 128, 0:128])
            nc.gpsimd.dma_start(bq_t[:], bq.rearrange("(c p) -> p c", p=128))
            nc.gpsimd.dma_start(bk_t[:], bk.rearrange("(c p) -> p c", p=128))
            for c in range(DC):
                nc.gpsimd.dma_start(wv[c][:], wvT[c * 128:(c + 1) * 128, :])
            for c in range(DC):
                nc.gpsimd.dma_start(wq[c][:, 128:D], wqT[c * 128:(c + 1) * 128, 128:D])
            for c in range(DC):
                nc.gpsimd.dma_start(wk[c][:, 128:D], wkT[c * 128:(c + 1) * 128, 128:D])
            # tiny dummy exp pulls the ~2.7us ACT table load off the
            # critical path (walrus emits the table load before the first
            # ACTIVATE in queue order)
            warm = const.tile([128, 1], F32, tag="warm")
            nc.scalar.activation(
                warm[:], mb_t[:, 0:1], mybir.ActivationFunctionType.Exp
            )

            # ---------- V storage [sc][128, H*65] fp32r ----------
            vaug = [const.tile([128, H * HW], BF16, tag=f"va{sc}", name=f"va{sc}") for sc in range(SC)]
            for sc in range(SC):
                ones_cols = vaug[sc][:].rearrange("p (h w) -> p h w", h=H)[:, :, HD:HW]
                nc.vector.memset(ones_cols, 1.0)

            # V projection piece, split into two 3-matmul chunks for
            # fine-grained injection between attention units
            def v_chunks(sc, half):
                n0, n1, h0, h1 = ((0, 512, 0, 8), (512, 768, 8, 12))[half]
                box = {}

                def c0():
                    box["vp"] = st_ps.tile([128, NT], F32, tag="st", name=f"vp{sc}_{half}")
                    for c in range(3):
                        nc.tensor.matmul(
                            box["vp"][:, : n1 - n0],
                            xt[c][:, sc * 128:(sc + 1) * 128],
                            wv[c][:, n0:n1],
                            start=(c == 0),
                            stop=False,
                            skip_group_check=True,
                        )

                def c1():
                    vp = box["vp"]
                    for c in range(3, DC):
                        nc.tensor.matmul(
                            vp[:, : n1 - n0],
                            xt[c][:, sc * 128:(sc + 1) * 128],
                            wv[c][:, n0:n1],
                            start=False,
                            stop=(c == DC - 1),
                            skip_group_check=True,
                        )
                    nc.vector.tensor_add(
                        vaug[sc][:].rearrange("p (h w) -> p h w", h=H)[:, h0:h1, 0:HD],
                        vp[:, : n1 - n0].rearrange("p (h w) -> p h w", w=HD),
                        bvb_t[:, n0:n1].rearrange("p (h w) -> p h w", w=HD),
                    )

                return [c0, c1]

            # ---------- Q/K projection ----------
            wmap = {"q": (wq, bq_t), "k": (wk, bk_t)}

            def qk_alloc(oc):
                return {
                    name: qk_pool.tile([128, S], BF16, tag=name, name=f"{name}t{oc}")
                    for name in ("q", "k")
                }

            def qk_chunks(oc, dsts, name, qt):
                w_t, b_t = wmap[name]
                box = {}

                def c0():
                    box["p"] = st_ps.tile([128, NT], F32, tag="st", name=f"qkp{name}{qt}")
                    for c in range(3):
                        nc.tensor.matmul(
                            box["p"][:],
                            w_t[c][:, oc * 128:(oc + 1) * 128],
                            xt[c][:, qt * NT:(qt + 1) * NT],
                            start=(c == 0),
                            stop=False,
                            skip_group_check=True,
                        )

                def c1():
                    p = box["p"]
                    for c in range(3, DC):
                        nc.tensor.matmul(
                            p[:],
                            w_t[c][:, oc * 128:(oc + 1) * 128],
                            xt[c][:, qt * NT:(qt + 1) * NT],
                            start=False,
                            stop=(c == DC - 1),
                            skip_group_check=True,
                        )
                    nc.vector.tensor_scalar_add(
                        dsts[name][:, qt * NT:(qt + 1) * NT], p[:], b_t[:, oc:oc + 1]
                    )

                return [c0, c1]

            def qk_proj(oc):
                dsts = qk_alloc(oc)
                for name in ("q", "k"):
                    for qt in range(QT):
                        for ch in qk_chunks(oc, dsts, name, qt):
                            ch()
                return dsts

            # ---------- attention: flat software pipeline, skew=2 ----------
            qkts = {0: qk_proj(0)}
            units = [(oc, hh, kc) for oc in range(OC) for hh in range(2)
                     for kc in range(SC)]
            NU = len(units)
            SKEW = 2
            st_tiles = {}
            pvq_map = {}

            def emit_scores(i):
                oc, hh, kc = units[i]
                p0 = hh * 64
                qkt = qkts[oc]
                stt = st_ps.tile([128, S], F32, tag="st", name=f"st{i}")
                for qt in range(QT):
                    nc.tensor.matmul(
                        stt[:, qt * NT:(qt + 1) * NT],
                        qkt["k"][p0:p0 + 64, kc * 128:(kc + 1) * 128],
                        qkt["q"][p0:p0 + 64, qt * NT:(qt + 1) * NT],
                        tile_position=(p0, 0),
                    )
                st_tiles[i] = stt

            def emit_epilogue(oc, hh):
                gh = 2 * oc + hh
                pvq = pvq_map.pop((oc, hh))
                pvs = epi_pool.tile([HW, S], F32, tag="pvs", name="pvs", bufs=3)
                # Z-row slivers first so the reciprocal round-trip overlaps
                # the big PV copies
                for qt in range(QT):
                    nc.vector.tensor_copy(
                        pvs[HD:HW, qt * NT:(qt + 1) * NT], pvq[qt][HD:HW, :]
                    )
                # Z row -> [128, 8] partition-scatter (p-major), reciprocal,
                # bounce through DRAM for the partition-broadcast read.
                # All DMAs ride the sync queue (no gpsimd semaphore hops).
                zp = epi_pool.tile([128, SC], F32, tag="zp", name="zp", bufs=4)
                nc.sync.dma_start(
                    zp[:], pvs[HD:HW, :].rearrange("o (p c) -> o p c", c=SC)
                )
                for qt in range(QT):
                    nc.vector.tensor_copy(
                        pvs[0:HD, qt * NT:(qt + 1) * NT], pvq[qt][0:HD, :]
                    )
                nc.vector.reciprocal(zp[:], zp[:])
                rzd = dram_pool.tile([S], F32, tag="rzd", name="rzd", bufs=4)
                nc.sync.dma_start(rzd.rearrange("(p c) -> p c", c=SC), zp[:])
                zb = epi_pool.tile([HD, S], F32, tag="zb", name="zb", bufs=3)
                nc.sync.dma_start(zb[:], rzd[:].partition_broadcast(HD))
                oh = epi_pool.tile([HD, S], F32, tag="oh", name="oh", bufs=3)
                nc.vector.tensor_mul(oh[:], pvs[0:HD, :], zb[:])
                nc.sync.dma_start(outT[gh * HD:(gh + 1) * HD, :], oh[:])

            # pre-roll: two score units, then V chunks for sc 0..1 so
            # exp(0)/exp(1) overlap the V projection start
            for i in range(SKEW):
                emit_scores(i)
            for sc in (0, 1):
                for half in (0, 1):
                    for ch in v_chunks(sc, half):
                        ch()

            # injection queue of small tensor-work chunks
            from collections import deque
            queue = deque()
            for sc in range(2, SC):
                for half in (0, 1):
                    queue.extend(v_chunks(sc, half))
            # rates[i] = how many chunks to inject after unit i
            rates = [0] * NU
            # 24 V chunks over units 0..5 (deadline: vaug[kc] before unit kc,
            # sc=2+j fully injected by end of unit j)
            for j in range(6):
                rates[j] += 4
            # Q/K proj for oc+1: 8 chunks over units (oc,0,6)..(oc,1,5) --
            # as late as the scores(+SKEW) deadline allows, to cover the
            # otherwise scalar-paced stretches
            qk_sched = {}
            for oc in range(OC - 1):
                base = oc * 16 + 6
                for j in range(8):
                    qk_sched.setdefault(base + j, []).append(oc + 1)
                    rates[base + j] += 1

            for i, (oc, hh, kc) in enumerate(units):
                if i + SKEW < NU:
                    emit_scores(i + SKEW)
                stt = st_tiles.pop(i)
                ett = et_pool.tile([128, S], BF16, tag="et", name=f"et{i}")
                nc.scalar.activation(
                    ett[:],
                    stt[:],
                    mybir.ActivationFunctionType.Exp,
                    bias=mb_t[:, kc:kc + 1],
                    scale=1.0 / np.sqrt(HD),
                )
                gh = 2 * oc + hh
                if kc == 0:
                    pvq_map[(oc, hh)] = [
                        pv_ps.tile([HW, NT], F32, tag="pv", name=f"pv{gh}_{qt}")
                        for qt in range(QT)
                    ]
                pvq = pvq_map[(oc, hh)]
                for qt in range(QT):
                    nc.tensor.matmul(
                        pvq[qt][:],
                        vaug[kc][:, gh * HW:(gh + 1) * HW],
                        ett[:, qt * NT:(qt + 1) * NT],
                        start=(kc == 0),
                        stop=(kc == SC - 1),
                    )
                if kc == SC - 1:
                    emit_epilogue(oc, hh)
                # inject queued projection chunks
                for oc_next in qk_sched.get(i, []):
                    if oc_next not in qkts:
                        qkts[oc_next] = qk_alloc(oc_next)
                        qkts.pop(oc_next - 2, None)
                        for name in ("q", "k"):
                            for qt in range(QT):
                                queue.extend(
                                    qk_chunks(oc_next, qkts[oc_next], name, qt)
                                )
                n = rates[i]
                while n > 0 and queue:
                    queue.popleft()()
                    n -= 1
            while queue:
                queue.popleft()()

    nc.compile()
    return nc


_NC = None


def _get_nc():
    global _NC
    if _NC is None:
        _NC = build()
    return _NC


def _in_maps(x, mask, Wq, bq, Wk, bk, Wv, bv):
    x = np.asarray(x, dtype=np.float32)
    mask = np.asarray(mask)
    bf = ml_dtypes.bfloat16
    wqT = np.ascontiguousarray(np.asarray(Wq, dtype=np.float32).T.astype(bf))
    wkT = np.ascontiguousarray(np.asarray(Wk, dtype=np.float32).T.astype(bf))
    wvT = np.ascontiguousarray(np.asarray(Wv, dtype=np.float32).T.astype(bf))
    bq = np.asarray(bq, dtype=np.float32)
    bk = np.asarray(bk, dtype=np.float32)
    bvb = np.ascontiguousarray(
        np.broadcast_to(np.asarray(bv, dtype=np.float32), (128, D))
    )
    maps = []
    for c in range(N_CORES):
        maps.append(
            {
                "xT": np.ascontiguousarray(x[c].T.astype(bf)),
                "wqT": wqT,
                "wkT": wkT,
                "wvT": wvT,
                "bq": bq,
                "bk": bk,
                "bvb": bvb,
                "mb": (-10000.0 * (1.0 - mask[c].astype(np.float32))).astype(
                    np.float32
                ),
            }
        )
    return maps


def run(inputs, trace=False, **kw):
    nc = _get_nc()
    res = run_bass_kernel_spmd(
        nc, _in_maps(**inputs), list(range(N_CORES)), trace=trace, **kw
    )
    out = np.stack(
        [np.ascontiguousarray(res.results[c]["outT"].T) for c in range(N_CORES)]
    ).astype(np.float32)
    return out, res


def kernel(**inputs):
    out, _ = run(inputs)
    return out
